# revision 15
# baseline (speedup 1.0000x reference)
"""Trainium2 Bass kernel for GQA attention (B=2, S=2048, DIM=2048, H=16, KV=8,
HD=128) with RoPE + causal mask + output projection.

Sharding: 8-way tensor parallelism over heads. Core c computes q heads
{2c, 2c+1} and kv head c end-to-end (QKV projection, RoPE, causal attention),
contributes its transposed attention output (256, 4096) to an on-device
AllGather, then computes the output-projection column slice
out[:, 256c:256(c+1)] = attn_full @ wo[:, 256c:256(c+1)] from the gathered
(2048, 4096) tensor. The host only slices inputs and concatenates outputs.

Layout tricks:
- Everything is computed transposed (feature dim on SBUF partitions) so no
  on-device transposes are needed anywhere except v (32 PE transposes).
- RoPE's interleaved (even, odd) pairs are handled by permuting wq/wk columns
  on the host to [evens, odds] per head, making the rotation act on two
  contiguous 64-partition halves. q/k are permuted consistently so q.k dot
  products are unchanged; v / wo stay unpermuted.
- Softmax runs in scoresT layout (keys on partitions): no max subtraction
  (scores are O(5) here), denominator via a ones-vector matmul, masking by
  multiplying exp by a 0/1 lower-triangle mask on diagonal tiles only.
- Matmuls are bf16 (fp32 accumulate); the 1/sqrt(HD) scale is folded into wq.
"""

import sys

if "/opt/trn_rl_repo" not in sys.path:
    sys.path.insert(0, "/opt/trn_rl_repo")

import numpy as np
import ml_dtypes

B, S, DIM = 2, 2048, 2048
H, KV, HD = 16, 8, 128
NC = 8
NS = B * S            # 4096 flattened (b, s) rows
P = 128
MB = DIM // P         # 16 contraction blocks for the projections
BF = ml_dtypes.bfloat16

_cache: dict = {}


def _build(debug=False):
    import concourse.bass as bass
    import concourse.mybir as mybir
    import concourse.tile as tile
    from concourse import bacc
    from concourse.masks import make_identity

    dt = mybir.dt
    f32, bf16 = dt.float32, dt.bfloat16
    Exp = mybir.ActivationFunctionType.Exp

    nc = bacc.Bacc("TRN2", debug=False, target_bir_lowering=False, num_devices=NC)

    xT_h = nc.dram_tensor("xT", (DIM, NS), bf16, kind="ExternalInput").ap()
    wq_h = nc.dram_tensor("wq_c", (DIM, 256), bf16, kind="ExternalInput").ap()
    wk_h = nc.dram_tensor("wk_c", (DIM, HD), bf16, kind="ExternalInput").ap()
    wv_h = nc.dram_tensor("wv_c", (DIM, HD), bf16, kind="ExternalInput").ap()
    wo_h = nc.dram_tensor("wo_c", (DIM, 256), bf16, kind="ExternalInput").ap()
    cos_h = nc.dram_tensor("cosT", (64, NS), f32, kind="ExternalInput").ap()
    sin_h = nc.dram_tensor("sinT", (64, NS), f32, kind="ExternalInput").ap()
    msk_h = nc.dram_tensor("masks", (P, 4 * 512), bf16, kind="ExternalInput").ap()
    out_h = nc.dram_tensor("outT", (256, NS), f32, kind="ExternalOutput").ap()
    dbg = {}
    if debug:
        bf16_ = __import__("concourse.mybir", fromlist=["dt"]).dt.bfloat16
        for nm, shp in [("qrot_d", (P, 2 * NS)), ("krot_d", (P, NS)),
                        ("vnat_d", (P, NS)), ("oav_d", (P, 2 * NS)),
                        ("ag_d", (NC * 256, NS))]:
            dbg[nm] = nc.dram_tensor(nm, shp, bf16_, kind="ExternalOutput").ap()

    with tile.TileContext(nc) as tc:
        with (
            tc.tile_pool(name="const", bufs=1) as const,
            tc.tile_pool(name="persist", bufs=1) as persist,
            tc.tile_pool(name="xs", bufs=4) as xs,
            tc.tile_pool(name="tmp", bufs=4) as tmp,
            tc.tile_pool(name="et", bufs=6) as et,
            tc.tile_pool(name="gs", bufs=4) as gs,
            tc.tile_pool(name="ot", bufs=3) as ot,
            tc.tile_pool(name="dram", bufs=1, space="DRAM") as dram,
        ):
            # ---- constants into SBUF ----
            wq_sb = const.tile([P, MB, 256], bf16)
            nc.sync.dma_start(wq_sb[:], wq_h.rearrange("(mb mi) d -> mi mb d", mi=P))
            wk_sb = const.tile([P, MB, HD], bf16)
            nc.sync.dma_start(wk_sb[:], wk_h.rearrange("(mb mi) d -> mi mb d", mi=P))
            wv_sb = const.tile([P, MB, HD], bf16)
            nc.sync.dma_start(wv_sb[:], wv_h.rearrange("(mb mi) d -> mi mb d", mi=P))
            wo_sb = const.tile([P, MB, 256], bf16)
            nc.sync.dma_start(wo_sb[:], wo_h.rearrange("(mb mi) d -> mi mb d", mi=P))
            cos_sb = const.tile([64, NS], f32)
            nc.sync.dma_start(cos_sb[:], cos_h)
            sin_sb = const.tile([64, NS], f32)
            nc.sync.dma_start(sin_sb[:], sin_h)
            msk_sb = const.tile([P, 4 * 512], bf16)
            nc.sync.dma_start(msk_sb[:], msk_h)
            ones_sb = const.tile([P, 1], bf16)
            nc.gpsimd.memset(ones_sb[:], 1.0)
            ones_row = const.tile([1, P], bf16)
            nc.gpsimd.memset(ones_row[:], 1.0)
            ident = const.tile([P, P], bf16)
            make_identity(nc, ident[:])

            # ---- persistent activations ----
            qrot = persist.tile([P, 2, NS], bf16)    # [d, head, s]
            krot = persist.tile([P, NS], bf16)
            vTt = persist.tile([P, NS], bf16)        # v transposed (d, s)
            vnat = persist.tile([P, NS // P, HD], bf16)  # [s_in, s_blk, d]
            oav = persist.tile([P, 2, NS], bf16)     # attention out (d, head, s)

            def rope_halves(pa, pb, cos_c, sin_c, out_even, out_odd):
                """(pa|pb) is a (128, 1024) psum pair; rotate into out slices."""
                for half, pp in ((0, pa), (1, pb)):
                    sl = slice(half * 512, (half + 1) * 512)
                    t1 = tmp.tile([64, 512], f32, tag="r1")
                    t2 = tmp.tile([64, 512], f32, tag="r2")
                    nc.vector.tensor_mul(t1[:], pp[0:64, :], cos_c[:, sl])
                    nc.vector.tensor_mul(t2[:], pp[64:128, :], sin_c[:, sl])
                    nc.vector.tensor_sub(out_even[:, sl], t1[:], t2[:])
                    t3 = tmp.tile([64, 512], f32, tag="r1")
                    t4 = tmp.tile([64, 512], f32, tag="r2")
                    nc.vector.tensor_mul(t3[:], pp[0:64, :], sin_c[:, sl])
                    nc.vector.tensor_mul(t4[:], pp[64:128, :], cos_c[:, sl])
                    nc.vector.tensor_add(out_odd[:, sl], t3[:], t4[:])

            # ---- phase 1: QKV projections (transposed), RoPE ----
            with tc.tile_pool(name="psA", bufs=1, space="PSUM") as psA:
                for sp in range(4):          # s windows of 1024
                    cw = slice(sp * 1024, (sp + 1) * 1024)
                    pq = [
                        [psA.tile([P, 512], f32, tag=f"pq{h}{a}", name=f"pq{h}{a}")
                         for a in range(2)]
                        for h in range(2)
                    ]
                    pk = [psA.tile([P, 512], f32, tag=f"pk{a}", name=f"pk{a}")
                          for a in range(2)]
                    pv = [psA.tile([P, 512], f32, tag=f"pv{a}", name=f"pv{a}")
                          for a in range(2)]
                    for m in range(MB):
                        xt = xs.tile([P, 1024], bf16, tag="xt")
                        nc.sync.dma_start(
                            xt[:], xT_h[m * P:(m + 1) * P, cw]
                        )
                        units = [
                            (pq[0], wq_sb[:, m, 0:128]),
                            (pq[1], wq_sb[:, m, 128:256]),
                            (pk, wk_sb[:, m, :]),
                            (pv, wv_sb[:, m, :]),
                        ]
                        for acc, lhsT in units:
                            for a in range(2):
                                nc.tensor.matmul(
                                    acc[a][:], lhsT, xt[:, a * 512:(a + 1) * 512],
                                    start=(m == 0), stop=(m == MB - 1),
                                )
                    cos_c, sin_c = cos_sb[:, cw], sin_sb[:, cw]
                    for h in range(2):
                        rope_halves(
                            pq[h][0], pq[h][1], cos_c, sin_c,
                            qrot[0:64, h, cw], qrot[64:128, h, cw],
                        )
                    rope_halves(pk[0], pk[1], cos_c, sin_c,
                                krot[0:64, cw], krot[64:128, cw])
                    for a in range(2):
                        nc.scalar.copy(
                            vTt[:, sp * 1024 + a * 512: sp * 1024 + (a + 1) * 512],
                            pv[a][:],
                        )

            # ---- phase 1b: v natural layout via PE transposes ----
            with tc.tile_pool(name="psT", bufs=2, space="PSUM") as psT:
                for blk in range(NS // P):
                    col = blk * P
                    pt = psT.tile([P, P], bf16, tag="pt")
                    nc.tensor.transpose(pt[:], vTt[:, col:col + P], ident[:])
                    nc.scalar.copy(vnat[:, blk, :], pt[:])

            # ---- phase 2: causal attention in scoresT layout ----
            with (
                tc.tile_pool(name="psS", bufs=2, space="PSUM") as psS,
                tc.tile_pool(name="psAcc", bufs=2, space="PSUM") as psAcc,
            ):
                for b in range(B):
                    for h in range(2):
                        for t in range(4):            # query chunks of 512
                            ic = slice(b * S + t * 512, b * S + (t + 1) * 512)
                            pden = psAcc.tile([1, 512], f32, tag="pden")
                            pav = psAcc.tile([P, 512], f32, tag="pav")
                            nj = 4 * t + 4
                            for j in range(nj):
                                ps = psS.tile([P, 512], f32, tag="ps")
                                jc = slice(b * S + j * P, b * S + (j + 1) * P)
                                nc.tensor.matmul(
                                    ps[:], krot[:, jc], qrot[:, h, ic],
                                    start=True, stop=True,
                                )
                                e = et.tile([P, 512], bf16, tag="e")
                                nc.scalar.activation(e[:], ps[:], Exp)
                                rel = j - 4 * t
                                if rel >= 0:
                                    nc.vector.tensor_mul(
                                        e[:], e[:],
                                        msk_sb[:, rel * 512:(rel + 1) * 512],
                                    )
                                nc.tensor.matmul(
                                    pden[:], ones_sb[:], e[:],
                                    start=(j == 0), stop=(j == nj - 1),
                                )
                                nc.tensor.matmul(
                                    pav[:], vnat[:, b * (S // P) + j, :], e[:],
                                    start=(j == 0), stop=(j == nj - 1),
                                )
                            rcp = tmp.tile([1, 512], f32, tag="rcp")
                            nc.vector.reciprocal(rcp[:], pden[:])
                            rcp_bf = tmp.tile([1, 512], bf16, tag="rcpc")
                            nc.vector.tensor_copy(rcp_bf[:], rcp[:])
                            rcp_ps = psAcc.tile([P, 512], f32, tag="rb")
                            nc.tensor.matmul(
                                rcp_ps[:], ones_row[:], rcp_bf[:],
                                start=True, stop=True,
                            )
                            rcp_b = tmp.tile([P, 512], f32, tag="rcpb")
                            nc.scalar.copy(rcp_b[:], rcp_ps[:])
                            nc.vector.tensor_mul(oav[:, h, ic], pav[:], rcp_b[:])

            # ---- phase 3: AllGather of transposed attention outputs ----
            ag_in = dram.tile([256, NS], bf16)
            ag_out = dram.tile([NC * 256, NS], bf16)
            for h in range(2):
                nc.sync.dma_start(ag_in[h * P:(h + 1) * P, :], oav[:, h, :])
            nc.gpsimd.collective_compute(
                "AllGather",
                mybir.AluOpType.bypass,
                replica_groups=[list(range(NC))],
                ins=[ag_in.opt()],
                outs=[ag_out.opt()],
            )

            if debug:
                nc.sync.dma_start(dbg["qrot_d"].rearrange("p (h s) -> p h s", h=2),
                                  qrot[:])
                nc.sync.dma_start(dbg["krot_d"], krot[:])
                nc.sync.dma_start(dbg["vnat_d"].rearrange("p (b d) -> p b d", b=NS // P),
                                  vnat[:])
                nc.sync.dma_start(dbg["oav_d"].rearrange("p (h s) -> p h s", h=2),
                                  oav[:])
                nc.sync.dma_start(dbg["ag_d"], ag_out[:])

            # ---- phase 4: output projection column slice ----
            with tc.tile_pool(name="psW", bufs=2, space="PSUM") as psW:
                for sp in range(4):          # s windows of 1024
                    pw = [
                        [psW.tile([P, 512], f32, tag=f"pw{n}{a}", name=f"pw{n}{a}")
                         for a in range(2)]
                        for n in range(2)
                    ]
                    for r in range(MB):
                        g = gs.tile([P, 1024], bf16, tag="g")
                        nc.sync.dma_start(
                            g[:], ag_out[r * P:(r + 1) * P, sp * 1024:(sp + 1) * 1024]
                        )
                        for n in range(2):
                            lhsT = wo_sb[:, r, n * 128:(n + 1) * 128]
                            for a in range(2):
                                nc.tensor.matmul(
                                    pw[n][a][:], lhsT, g[:, a * 512:(a + 1) * 512],
                                    start=(r == 0), stop=(r == MB - 1),
                                )
                    for n in range(2):
                        for a in range(2):
                            o = ot.tile([P, 512], f32, tag="o")
                            nc.scalar.copy(o[:], pw[n][a][:])
                            nc.sync.dma_start(
                                out_h[n * P:(n + 1) * P,
                                      sp * 1024 + a * 512: sp * 1024 + (a + 1) * 512],
                                o[:],
                            )

    nc.compile()
    return nc


def _prep_inputs(x, freqs_cos, freqs_sin, wq, wk, wv, wo):
    x = np.asarray(x, np.float32).reshape(NS, DIM)
    xT = np.ascontiguousarray(x.T).astype(BF)
    cos = np.asarray(freqs_cos, np.float32)
    sin = np.asarray(freqs_sin, np.float32)
    cosT = np.ascontiguousarray(np.tile(cos, (B, 1)).T)
    sinT = np.ascontiguousarray(np.tile(sin, (B, 1)).T)

    perm = np.r_[np.arange(0, HD, 2), np.arange(1, HD, 2)]
    scale = np.float32(1.0 / np.sqrt(HD))
    wq = np.asarray(wq, np.float32) * scale
    wk = np.asarray(wk, np.float32)
    wv = np.asarray(wv, np.float32)
    wo = np.asarray(wo, np.float32)

    masks = np.zeros((P, 4, 512), np.float32)
    for p in range(4):
        for isub in range(4):
            sl = slice(isub * 128, (isub + 1) * 128)
            if p < isub:
                masks[:, p, sl] = 1.0
            elif p == isub:
                masks[:, p, sl] = np.triu(np.ones((P, P), np.float32))
    masks = np.ascontiguousarray(masks.reshape(P, 4 * 512)).astype(BF)

    in_maps = []
    for c in range(NC):
        wq_c = wq[:, c * 256:(c + 1) * 256]
        wq_cp = np.concatenate([wq_c[:, h * HD + perm] for h in range(2)], axis=1)
        in_maps.append({
            "xT": xT,
            "wq_c": np.ascontiguousarray(wq_cp).astype(BF),
            "wk_c": np.ascontiguousarray(wk[:, c * HD:(c + 1) * HD][:, perm]).astype(BF),
            "wv_c": np.ascontiguousarray(wv[:, c * HD:(c + 1) * HD]).astype(BF),
            "wo_c": np.ascontiguousarray(wo[:, c * 256:(c + 1) * 256]).astype(BF),
            "cosT": cosT,
            "sinT": sinT,
            "masks": masks,
        })
    return in_maps


def _run(inputs, trace=False, **kw):
    from concourse.bass_utils import run_bass_kernel_spmd

    if "nc" not in _cache:
        _cache["nc"] = _build()
    nc = _cache["nc"]
    in_maps = _prep_inputs(**inputs)
    res = run_bass_kernel_spmd(
        nc, in_maps, core_ids=list(range(NC)), trace=trace, **kw
    )
    out = np.empty((NS, DIM), np.float32)
    for c in range(NC):
        out[:, c * 256:(c + 1) * 256] = res.results[c]["outT"].T
    return out.reshape(B, S, DIM), res


def kernel(**inputs) -> np.ndarray:
    out, _ = _run(inputs, trace=False)
    return out
